# revision 6
# baseline (speedup 1.0000x reference)
"""Trainium2 Bass kernel for nn_Classifier (segment_reduce).

Computation (reference):
    local  = relu(x @ W1.T)            # [T, 50] @ [50, 400] -> [T, 400]
    feat   = mean over windows of J=24 # [T//24, 400]
    logits = feat @ W2.T               # [T//24, 400] @ [400, 10]

Strategy: pure data parallel over 8 NeuronCores (x sharded along T).
Per core (T_c = 98304 rows = 4096 windows):
  - Host packs the x shard TRANSPOSED + bf16 into xp [100, 49152]:
    rows 0-49 hold x_shard[:49152].T, rows 50-99 hold x_shard[49152:].T.
    On-chip it lands at SBUF partitions 0-49 / 64-113 so the two shard
    halves row-tile the PE array (tile_position (0,0)/(64,0)) and the
    hh matmul pairs run concurrently on disjoint row groups.
  - matmul1: lhsT = xp[, 128-col tile] (stationary), rhs = W1.T [50, 400]
    -> psum [128t, 400k] fp32; two tiles packed per [128, 1024] psum pair.
  - relu evacuation psum->sbuf bf16 split across ScalarE (Relu, 13/24)
    and VectorE (tensor_scalar_max, 11/24) - the throughput-limiting
    stage (~160us/core); everything else is scheduled to hide under it.
  - pooling runs on the PE: stationary 0/1 matrices contract 24-row
    windows across psum partitions. Pooling matmuls are batched per
    half-supergroup (12 pairs), delayed one half-batch behind mm1, and
    interleaved across the 4 column strips of the array so 4 run
    concurrently; they accumulate into a shared feat psum per shard-half.
  - the per-supergroup tail (feat evac, transpose-matmuls that also
    unscramble window order, matmul2, logits out) is deferred into the
    next supergroup's pair stream so it never stalls the evac engines.
"""

import sys

sys.path.insert(0, "/opt/trn_rl_repo")

import numpy as np
import ml_dtypes

import bass_rust
import concourse.bass as bass
import concourse.mybir as mybir
import concourse.tile as tile
from concourse.bass_utils import run_bass_kernel_spmd
from concourse.tile import TileContext
from concourse.vector_clock import ScopedClock

# ---------------------------------------------------------------------------
# Wait-count legalization (monkeypatch).
#
# This walrus build accepts at most 1 sync-wait per instruction (2 for
# EventSemaphore), but Tile's scheduler and tail drain can attach more,
# failing codegen with "Too many sync wait commands". Spread excess waits
# onto same-engine NOPs inserted immediately before the instruction.
# ---------------------------------------------------------------------------

_orig_add = TileContext._add_instruction


def _wait_cap(inst):
    return 2 if type(inst).__name__ == "InstEventSemaphore" else 1


def _patched_add_instruction(self, inst):
    si = inst.sync_info
    cap = _wait_cap(inst)
    if (
        si is not None
        and si.on_wait
        and len(si.on_wait) > cap
        and inst.engine != mybir.EngineType.Unassigned
    ):
        waits = list(si.on_wait)
        for w in waits[:-cap]:
            nop = bass_rust.InstNoOp(
                name=f"I-waitfix-{self.nc.next_id()}",
                opcode="NoOp",
                engine=inst.engine,
                ins=[],
                outs=[],
            )
            nop.sync_info = mybir.SyncInfo(on_wait=[w], on_update=[])
            _orig_add(self, nop)
        inst.sync_info = mybir.SyncInfo(
            on_wait=waits[-cap:], on_update=list(si.on_update or [])
        )
    _orig_add(self, inst)


def _patched_drain_and_barrier(self, tick_clock, wait_clock):
    nc = self.nc
    drain_inst = nc.sync.drain()
    wait_clock.add_sem_waits(
        drain_inst.ins, ScopedClock({None: tick_clock.global_clock})
    )
    mi = drain_inst.ins
    si = mi.sync_info
    waits = list(si.on_wait) if (si and si.on_wait) else []
    if len(waits) > 1:
        mi.sync_info = mybir.SyncInfo(
            on_wait=[waits[-1]], on_update=list(si.on_update or [])
        )
        for w in waits[:-1]:
            nop = nc.sync.nop()
            nop.ins.sync_info = mybir.SyncInfo(on_wait=[w], on_update=[])

    nc.all_engine_barrier()
    assert self.sems is not None
    popped = nc._tile_sem_poison_stack.pop()
    assert popped is self._sem_poison
    nc.clear_and_free_semaphores(list(self.sems.allocated().values()))
    nc.all_engine_barrier()


TileContext._add_instruction = _patched_add_instruction
TileContext._drain_and_barrier = _patched_drain_and_barrier

# ---------------------------------------------------------------------------
# Problem constants (hardcoded per the harness contract)
# ---------------------------------------------------------------------------

J = 24
T, N, K, C = 786432, 50, 400, 10
NCORES = 8
TC = T // NCORES          # 98304 rows per core
H = TC // 2               # 49152 cols per half in xp
B_CORE = TC // J          # 4096 windows per core
NG = 16                   # supergroup iterations (8 groups x 3 tiles each)
CHUNK = 24 * 128          # 3072 xp columns per supergroup

BF16 = mybir.dt.bfloat16
F32 = mybir.dt.float32
nbf = ml_dtypes.bfloat16


def _build_pmats():
    """P[j, q] in one [128, 192] array: col block (j*2+q)*32.
    P[j,q][tau, 16*q + w] = 1 where window w = (128*j + tau) // 24 of the
    384-row group; q = which half of the 32-partition strip."""
    pm = np.zeros((128, 192), np.float32)
    for j_ in range(3):
        for q in range(2):
            base = (j_ * 2 + q) * 32
            for tau in range(128):
                w = (128 * j_ + tau) // 24
                pm[tau, base + 16 * q + w] = 1.0
    return pm.astype(nbf)


def _build_permmat():
    """Permutation for the feat transpose: featT column m takes feat row
    p = 32*s + 16*q + w where m = 64*q + 16*s + w (un-scrambles the
    pooling strip layout into natural window order)."""
    pm = np.zeros((128, 128), np.float32)
    for m in range(128):
        w = m % 16
        s = (m // 16) % 4
        q = m // 64
        p = 32 * s + 16 * q + w
        pm[p, m] = 1.0
    return pm.astype(nbf)


def _build_nc(repeat: int = 1):
    """repeat>1 re-runs the whole computation in one NEFF — used by the
    test harness to measure device time differentially (wall(R=hi) -
    wall(R=1))/(hi-1) without NTFF profiling."""
    nc = bass.Bass()
    xp_d = nc.declare_dram_parameter("xp", [100, H], BF16, isOutput=False)
    w1t_d = nc.declare_dram_parameter("w1t", [50, 400], BF16, isOutput=False)
    w2tp_d = nc.declare_dram_parameter("w2tp", [100, 40], BF16, isOutput=False)
    pm_d = nc.declare_dram_parameter("pmats", [128, 192], BF16, isOutput=False)
    perm_d = nc.declare_dram_parameter("perm", [128, 128], BF16, isOutput=False)
    out_d = nc.declare_dram_parameter("logits", [B_CORE, 10], F32, isOutput=True)

    act = mybir.ActivationFunctionType

    with TileContext(nc) as tc:
        with (
            tc.tile_pool(name="consts", bufs=1) as cpool,
            tc.tile_pool(name="xchunks", bufs=2) as xpool,
            tc.tile_pool(name="relu", bufs=26) as rpool,
            tc.tile_pool(name="small", bufs=6) as spool,
            tc.tile_pool(name="mm1ps", bufs=2, space="PSUM") as mm1pool,
            tc.tile_pool(name="featps", bufs=2, space="PSUM") as featpool,
            tc.tile_pool(name="tailps", bufs=1, space="PSUM") as tailpool,
        ):
            # W1T staged at partition offsets 0 and 64 — the moving operand
            # must share the stationary's base partition (array row offset).
            w1t = cpool.tile([128, 400], BF16)
            w2tp = cpool.tile([100, 40], BF16)
            pmats = cpool.tile([128, 192], BF16)
            perm = cpool.tile([128, 128], BF16)
            nc.sync.dma_start(out=w1t[0:50, :], in_=w1t_d[:])
            nc.sync.dma_start(out=w1t[64:114, :], in_=w1t_d[:])
            nc.sync.dma_start(out=w2tp[:], in_=w2tp_d[:])
            nc.sync.dma_start(out=pmats[:], in_=pm_d[:])
            nc.sync.dma_start(out=perm[:], in_=perm_d[:])

            # ---- deferred emitters -------------------------------------
            featps_live = {}

            # pool-batch: pooling matmuls for one half-supergroup (12
            # pairs), strip-interleaved so 4 run concurrently on the PE.
            # featps for the supergroup is allocated here (h==0) so the
            # WAR on the psum slots is emitted after the previous group's
            # feat evacuation (its reader).
            def emit_pool_batch(rls, Gkey, h):
                # rls[3*s + r] is pair p = 12*h + 3*s + r; q = h
                if h == 0:
                    featps_live[Gkey] = [
                        featpool.tile([128, 400], F32, name="featps")
                        for _ in range(2)
                    ]
                featps = featps_live[Gkey]
                q = h
                for r in range(3):
                    for hh in range(2):
                        for s in range(4):
                            rl = rls[3 * s + r]
                            first = r == 0 and q == 0
                            last = r == 2 and q == 1
                            nc.tensor.matmul(
                                featps[hh][32 * s : 32 * s + 32, :],
                                pmats[:, (r * 2 + q) * 32 : (r * 2 + q) * 32 + 32],
                                rl[:, hh, :],
                                start=first,
                                stop=last,
                                # auto-derive rejects base partition 96
                                tile_position=(0, 32 * s),
                            )

            # tail: feat evac was already emitted; transpose + matmul2 +
            # logits out for supergroup G.
            def emit_tail(G, feats):
                for hh in range(2):
                    feat = feats[hh]
                    ftps = tailpool.tile([100, 512], BF16, name="ftps")
                    for c in range(4):
                        nc.tensor.matmul(
                            ftps[:, 128 * c : 128 * (c + 1)],
                            feat[:, 100 * c : 100 * (c + 1)],
                            perm[:],
                            is_transpose=True,
                            start=(c == 0),
                            stop=(c == 3),
                        )
                    ft = spool.tile([100, 512], BF16, name="ft")
                    nc.vector.tensor_copy(out=ft[:], in_=ftps[:])

                    lps = tailpool.tile([128, 16], F32, name="lps")
                    for c in range(4):
                        nc.tensor.matmul(
                            lps[:, 0:10],
                            ft[:, 128 * c : 128 * (c + 1)],
                            w2tp[:, 10 * c : 10 * (c + 1)],
                            start=(c == 0),
                            stop=(c == 3),
                        )
                    lsb = spool.tile([128, 10], F32, name="lsb")
                    nc.vector.tensor_copy(out=lsb[:], in_=lps[:, 0:10])
                    rowbase = hh * (B_CORE // 2) + G * 128
                    nc.sync.dma_start(
                        out=out_d[rowbase : rowbase + 128, :], in_=lsb[:]
                    )

            # ---- main software pipeline over half-batches ---------------
            HBs = [(g, h) for _ in range(repeat) for g in range(NG) for h in (0, 1)]
            evac_ct = 0
            pend_pool = None   # (rls, gi, h) awaiting pooling emission
            pend_feat = None   # (G, gi) awaiting feat evacuation
            pend_tail = None   # (G, feats) awaiting tail emission
            xc = None

            def emit_feat_evac(pend):
                pG, gi = pend
                pfeatps = featps_live.pop(gi)
                feats = []
                for hh in range(2):
                    feat = spool.tile([128, 400], BF16, name="feat")
                    nc.scalar.activation(feat[:], pfeatps[hh][:], act.Relu)
                    feats.append(feat)
                return (pG, feats)

            for idx, (G, h) in enumerate(HBs):
                gi = idx // 2
                if h == 0:
                    xc = xpool.tile([128, CHUNK], BF16, name="xc")
                    # packed xp: rows 0-49 -> partitions 0-49,
                    # rows 50-99 -> partitions 64-113 (no zero rows on the
                    # wire)
                    nc.sync.dma_start(
                        out=xc[0:50, :], in_=xp_d[0:50, G * CHUNK : (G + 1) * CHUNK]
                    )
                    nc.sync.dma_start(
                        out=xc[64:114, :],
                        in_=xp_d[50:100, G * CHUNK : (G + 1) * CHUNK],
                    )

                cur_rls = []
                for pp in range(12):
                    p = 12 * h + pp
                    tcol = p * 128
                    ps = mm1pool.tile([128, 1024], F32, name="ps")
                    for hh in range(2):
                        rb = 64 * hh
                        nc.tensor.matmul(
                            ps[:, 512 * hh : 512 * hh + 400],
                            xc[rb : rb + 50, tcol : tcol + 128],
                            w1t[rb : rb + 50, :],
                            start=True,
                            stop=True,
                        )
                    # evacuate the pair (both halves) in one op
                    rl = rpool.tile([128, 2, 400], BF16, name="rl", bufs=26)
                    src = ps[:, :].rearrange("p (two k) -> p two k", two=2)[
                        :, :, 0:400
                    ]
                    # measured: ACT 767ns vs DVE 850ns per pair -> 13:11
                    if evac_ct % 24 in (0, 2, 4, 6, 9, 11, 13, 15, 17, 19, 21):
                        nc.vector.tensor_scalar_max(rl[:], src, 0.0)
                    else:
                        nc.scalar.activation(rl[:], src, act.Relu)
                    evac_ct += 1
                    cur_rls.append(rl)

                    # slot deferred work into the pair stream so the PE/
                    # engines never sit idle at batch boundaries
                    if pp == 2 and pend_pool is not None:
                        emit_pool_batch(*pend_pool)
                        pend_pool = None
                        if pend_feat is not None:
                            # pooling for G-1 fully emitted -> feat evac
                            pend_tail = emit_feat_evac(pend_feat)
                            pend_feat = None
                    if pp == 6 and pend_tail is not None:
                        emit_tail(*pend_tail)
                        pend_tail = None

                pend_pool = (cur_rls, gi, h)
                if h == 1:
                    pend_feat = (G, gi)

            # epilogue: drain the pipeline
            if pend_pool is not None:
                emit_pool_batch(*pend_pool)
            if pend_feat is not None:
                pend_tail = emit_feat_evac(pend_feat)
            if pend_tail is not None:
                emit_tail(*pend_tail)
    return nc


_NC = {}


def _get_nc(repeat: int = 1):
    if repeat not in _NC:
        _NC[repeat] = _build_nc(repeat)
    return _NC[repeat]


def prepare_in_maps(x: np.ndarray, W1: np.ndarray, W2: np.ndarray):
    assert x.shape == (T, N) and W1.shape == (K, N) and W2.shape == (C, K)

    w1t = np.ascontiguousarray(W1.T.astype(nbf))          # [50, 400]
    w2tp = np.ascontiguousarray(
        (W2.T.astype(np.float32) / J).reshape(4, 100, 10).transpose(1, 0, 2)
        .reshape(100, 40)
    ).astype(nbf)                                          # [100, 4*10]
    pmats = _build_pmats()
    permm = _build_permmat()

    xb = x.astype(nbf)
    in_maps = []
    for c in range(NCORES):
        shard = xb[c * TC : (c + 1) * TC]                  # [98304, 50]
        xp = np.empty((100, H), nbf)
        xp[0:50] = shard[0:H].T
        xp[50:100] = shard[H:].T
        in_maps.append(
            {
                "xp": xp,
                "w1t": w1t,
                "w2tp": w2tp,
                "pmats": pmats,
                "perm": permm,
            }
        )
    return in_maps


def kernel(x: np.ndarray, W1: np.ndarray, W2: np.ndarray) -> np.ndarray:
    in_maps = prepare_in_maps(x, W1, W2)
    nc = _get_nc()
    res = run_bass_kernel_spmd(nc, in_maps, core_ids=list(range(NCORES)))
    out = np.concatenate(
        [res.results[c]["logits"] for c in range(NCORES)], axis=0
    )
    return out.astype(np.float32)
